# revision 5
# baseline (speedup 1.0000x reference)
"""Trainium2 Bass kernel for the CAB fusion:

    out = shallower * sigmoid(MLP(concat(gap(shallower), gap(deeper)))) +
          bilinear_upsample_2x(deeper)

Sharding: pure data parallel - batch 16 split 2-per-core across 8
NeuronCores; tiny 1x1-conv weights replicated.

v2 design (from the v1 trace: DVE 47us busy fp32, PE 65us occupied by
96 LDWEIGHTS+MATMUL pairs, DMA 50us for 19.7MB, 11us of DVE drains from
accum_out reads):
  - bf16 on the whole data path (host-side cast; rel-err budget 2e-2).
    Halves HBM traffic to ~9.4MB/core and gives DVE 2x packed ops on
    unit-stride tensor_tensor work.
  - deeper's global pool is taken directly from the (1/16-prescaled)
    deeper input tile on ScalarE right after its DMA lands - no DVE
    accum_out (drains), and the gate no longer waits on the upsample.
    Fold: mean(deeper) = sum(d16)/64 -> w1 deeper rows get a 1/64 host
    scale (shallow rows keep 1/4096).
  - MLP: pool pieces per group packed as columns of one tile; layer 1 is
    4 deeper-chunk matmuls issued early + 4 shallow-chunk matmuls into
    the same PSUM accumulation group; PSUM columns are summed by a
    ScalarE Copy+accum_out, ReLU'd on ScalarE (warmed table). 12
    LD+MM pairs per batch instead of 48.
  - upsample W-pass edge cols and H-pass edge rows are single-source
    ops -> ScalarE activation Copy with scale=4 (off the DVE).
  - deeper + weights ride the GpSimd/SWDGE DMA queue, shallower halves
    split across the two HWDGE rings (sync + scalar/ACT), so all three
    descriptor generators run in parallel from t=0.
  - outputs stored bf16 in an SBUF-mirroring packed layout (large
    descriptors); host unpacks to [B,C,H,W] fp32.
"""

import numpy as np
import ml_dtypes
from contextlib import ExitStack

import concourse.bacc as bacc
import concourse.tile as tile
import concourse.mybir as mybir
from concourse import bass_utils

F32 = mybir.dt.float32
BF16 = mybir.dt.bfloat16
AF = mybir.ActivationFunctionType
OP = mybir.AluOpType

N_CORES = 8
B, C = 16, 256
HD, WD = 32, 32
HS, WS = 64, 64
BL = B // N_CORES          # batches per core
G = C // 128               # channel groups of 128
DHW = HD * WD              # 1024
SHW = HS * WS              # 4096
CP = 6 * C + 4             # packed weights+bias columns


def _emit(ctx, tc, dpk, spk, cpack, out):
    nc = tc.nc

    wpool = ctx.enter_context(tc.tile_pool(name="weights", bufs=1))
    stat = ctx.enter_context(tc.tile_pool(name="stat", bufs=1))
    sres = ctx.enter_context(tc.tile_pool(name="sres", bufs=1))
    up = ctx.enter_context(tc.tile_pool(name="up", bufs=2))
    ures = ctx.enter_context(tc.tile_pool(name="ures", bufs=4))
    psum = ctx.enter_context(tc.tile_pool(name="psum", bufs=1, space="PSUM"))

    dpk_v = dpk.rearrange("p (b g h w) -> p b g h w", b=BL, g=G, h=HD)
    spk_v = spk.rearrange("p (b g h w) -> p b g h w", b=BL, g=G, h=HS)
    out_v = out.rearrange("p (b g h w) -> p b g h w", b=BL, g=G, h=HS)

    # ---- loads.  SWDGE (gpsimd) queue: deeper per (b,g) then weights.
    # HWDGE rings: shallower tiles split in halves, one half per ring.
    dsb = wpool.tile([128, BL, G, HD, WD], BF16, name="dsb")
    for b in range(BL):
        for g in range(G):
            nc.gpsimd.dma_start(dsb[:, b, g], dpk_v[:, b, g])
    cpk_sb = wpool.tile([128, CP], F32, name="cpk_sb")
    nc.gpsimd.dma_start(cpk_sb[:], cpack[:, :])
    wmat = cpk_sb[:, 0:6 * C].rearrange("p (k o) -> p k o", k=6)
    bias_sb = cpk_sb[:, 6 * C:]

    half = HS // 2
    s_sb = {}
    for b in range(BL):
        for g in range(G):
            st = sres.tile([128, HS, WS], BF16, name=f"s{b}{g}")
            nc.sync.dma_start(st[:, 0:half, :], spk_v[:, b, g, 0:half, :])
            nc.scalar.dma_start(st[:, half:HS, :], spk_v[:, b, g, half:HS, :])
            s_sb[b, g] = st

    # ---- warm the ACT tables (Relu for layer-1, Sigmoid for the gate)
    # before the pools need them; after all load triggers.
    warm = stat.tile([128, 1], F32, name="warm")
    nc.vector.memset(warm[:], 0.0)
    nc.scalar.activation(warm[:], warm[:], AF.Relu)
    nc.scalar.activation(warm[:], warm[:], AF.Sigmoid)

    # ---- pool pieces per group: cols (b*4 + [dpool, zero, sp_half0,
    # sp_half1]).  The zero column makes the early deeper-chunk matmul
    # full-width so start=True zeroes the whole PSUM accumulation region.
    piece = [stat.tile([128, 4 * BL], F32, name=f"piece{g}") for g in range(G)]
    for g in range(G):
        nc.gpsimd.memset(piece[g][:], 0.0)
    hsum = [stat.tile([128, BL], F32, name=f"hsum{og}") for og in range(G)]
    hcol = [stat.tile([128, BL], F32, name=f"hcol{og}") for og in range(G)]
    sig = [stat.tile([128, BL], BF16, name=f"sig{g}") for g in range(G)]

    # deeper pools: in-place Copy+accum right after each deeper tile lands
    for b in range(BL):
        for g in range(G):
            d = dsb[:, b, g]
            nc.scalar.activation(d, d, AF.Copy,
                                 accum_out=piece[g][:, 4 * b:4 * b + 1])

    # layer-1 deeper chunks: start the PSUM accumulation groups early
    # (only need dsb + weights).  ph[og][b] accumulates into column 0.
    ph = [[psum.tile([128, 2], F32, name=f"ph{og}{b}") for b in range(BL)]
          for og in range(G)]
    for b in range(BL):
        for og in range(G):
            ogs = slice(og * 128, (og + 1) * 128)
            for g in range(G):
                nc.tensor.matmul(ph[og][b][:, 0:2], wmat[:, 2 + g, ogs],
                                 piece[g][:, 4 * b:4 * b + 2],
                                 start=(g == 0), stop=False)

    # ---- upsample (per (b,g)): W-pass 32x32 -> yp 32x64, H-pass -> u
    # 64x64.  Interior on DVE, edges on ScalarE (single-source, scale=4).
    u_sb = {}

    def upsample(b, g):
        d = dsb[:, b, g]
        yp = up.tile([128, HD, WS], BF16, name="yp")
        ypv = yp.rearrange("p h (j t) -> p h j t", t=2)
        nc.vector.scalar_tensor_tensor(
            ypv[:, :, 1:WD, 0], d[:, :, 1:WD], 3.0, d[:, :, 0:WD - 1],
            OP.mult, OP.add)
        nc.vector.scalar_tensor_tensor(
            ypv[:, :, 0:WD - 1, 1], d[:, :, 0:WD - 1], 3.0, d[:, :, 1:WD],
            OP.mult, OP.add)
        # W edge cols 0 and 63 on ScalarE
        nc.scalar.activation(yp[:, :, 0:WS:WS - 1], d[:, :, 0:WD:WD - 1],
                             AF.Copy, scale=4.0)

        u = ures.tile([128, HS, WS], BF16, name="u")
        uv = u.rearrange("p (i t) w -> p i t w", t=2)
        nc.vector.scalar_tensor_tensor(
            uv[:, 1:HD, 0, :], yp[:, 1:HD, :], 3.0, yp[:, 0:HD - 1, :],
            OP.mult, OP.add)
        nc.vector.scalar_tensor_tensor(
            uv[:, 0:HD - 1, 1, :], yp[:, 0:HD - 1, :], 3.0, yp[:, 1:HD, :],
            OP.mult, OP.add)
        # H edge rows 0 and 63 on ScalarE
        nc.scalar.activation(u[:, 0:HS:HS - 1, :], yp[:, 0:HD:HD - 1, :],
                             AF.Copy, scale=4.0)
        u_sb[b, g] = u

    # shallower pools (per loaded half, ScalarE in-place Copy+accum)
    def s_pool(b):
        for g in range(G):
            st = s_sb[b, g]
            nc.scalar.activation(st[:, 0:half, :], st[:, 0:half, :], AF.Copy,
                                 accum_out=piece[g][:, 4 * b + 2:4 * b + 3])
            nc.scalar.activation(st[:, half:HS, :], st[:, half:HS, :], AF.Copy,
                                 accum_out=piece[g][:, 4 * b + 3:4 * b + 4])

    # finish the MLP for batch b: shallow layer-1 chunks, PSUM-column
    # sum + ReLU on ScalarE, layer 2, sigmoid.
    def mlp(b):
        for og in range(G):
            ogs = slice(og * 128, (og + 1) * 128)
            for g in range(G):
                nc.tensor.matmul(ph[og][b][:, 0:2], wmat[:, g, ogs],
                                 piece[g][:, 4 * b + 2:4 * b + 4],
                                 start=False, stop=(g == G - 1))
        for og in range(G):
            p = ph[og][b]
            nc.scalar.activation(p[:], p[:], AF.Copy,
                                 accum_out=hsum[og][:, b:b + 1])
            nc.scalar.activation(hcol[og][:, b:b + 1], hsum[og][:, b:b + 1],
                                 AF.Relu, bias=bias_sb[:, og:og + 1])
        for g2 in range(G):
            g2s = slice(g2 * 128, (g2 + 1) * 128)
            pg = psum.tile([128, 1], F32, name=f"pg{g2}{b}")
            for ig in range(G):
                nc.tensor.matmul(pg[:], wmat[:, 4 + ig, g2s],
                                 hcol[ig][:, b:b + 1],
                                 start=(ig == 0), stop=(ig == G - 1))
            nc.scalar.activation(sig[g2][:, b:b + 1], pg[:], AF.Sigmoid,
                                 bias=bias_sb[:, 2 + g2:3 + g2])

    # finals for (b,g): out = s * sig + u, stored in row chunks that
    # alternate HWDGE rings.  `bounds` tapers at the very end so the
    # trailing store after the last DVE op is short.
    store_flip = [0]

    def finals(b, g, bounds):
        s = s_sb[b, g]
        u = u_sb[b, g]
        sc = sig[g][:, b:b + 1]
        for q in range(len(bounds) - 1):
            rows = slice(bounds[q], bounds[q + 1])
            nc.vector.scalar_tensor_tensor(
                s[:, rows, :], s[:, rows, :], sc, u[:, rows, :],
                OP.mult, OP.add)
            eng = nc.sync if store_flip[0] % 2 == 0 else nc.scalar
            store_flip[0] += 1
            eng.dma_start(out_v[:, b, g, rows, :], s[:, rows, :])

    # ---- schedule ----
    upsample(0, 0)
    upsample(0, 1)
    s_pool(0)
    mlp(0)
    upsample(1, 0)
    s_pool(1)
    mlp(1)
    finals(0, 0, [0, 32, 64])
    finals(0, 1, [0, 32, 64])
    upsample(1, 1)
    finals(1, 0, [0, 16, 32, 48, 64])
    finals(1, 1, [0, 16, 32, 48, 56, 64])


def build_kernel():
    nc = bacc.Bacc("TRN2", target_bir_lowering=False, debug=False,
                   num_devices=N_CORES)
    dpk = nc.dram_tensor("dpk", [128, BL * G * DHW], BF16,
                         kind="ExternalInput").ap()
    spk = nc.dram_tensor("spk", [128, BL * G * SHW], BF16,
                         kind="ExternalInput").ap()
    cpack = nc.dram_tensor("cpack", [128, CP], F32, kind="ExternalInput").ap()
    out = nc.dram_tensor("out", [128, BL * G * SHW], BF16,
                         kind="ExternalOutput").ap()

    with tile.TileContext(nc) as tc, ExitStack() as ctx:
        _emit(ctx, tc, dpk, spk, cpack, out)
    nc.compile()
    return nc


_NC = None


def _get_nc():
    global _NC
    if _NC is None:
        _NC = build_kernel()
    return _NC


def prepare_in_maps(deeper, shallower, w1, b1, w2, b2):
    # w1t transposed: shallow rows fold the 1/(64*64) mean; deeper rows
    # fold mean(deeper) = sum(deeper/16)/64.
    w1t = np.ascontiguousarray(np.asarray(w1).T).astype(np.float32)  # [512,256]
    w1t[0:C] *= np.float32(1.0 / 4096.0)
    w1t[C:2 * C] *= np.float32(1.0 / 64.0)
    w2t = np.ascontiguousarray(np.asarray(w2).T).astype(np.float32)  # [256,256]
    wp = np.empty((128, CP), np.float32)
    for k in range(4):
        wp[:, k * C:(k + 1) * C] = w1t[k * 128:(k + 1) * 128]
    for k in range(2):
        wp[:, (4 + k) * C:(5 + k) * C] = w2t[k * 128:(k + 1) * 128]
    b1f = np.asarray(b1, np.float32).reshape(2, 128)
    b2f = np.asarray(b2, np.float32).reshape(2, 128)
    wp[:, 6 * C + 0] = b1f[0]
    wp[:, 6 * C + 1] = b1f[1]
    wp[:, 6 * C + 2] = b2f[0]
    wp[:, 6 * C + 3] = b2f[1]

    d16 = (np.asarray(deeper, np.float32) * np.float32(1.0 / 16.0)).astype(
        ml_dtypes.bfloat16)
    sbf = np.asarray(shallower, np.float32).astype(ml_dtypes.bfloat16)
    in_maps = []
    for i in range(N_CORES):
        dc = d16[i * BL:(i + 1) * BL].reshape(BL, G, 128, DHW)
        dpk = np.ascontiguousarray(
            dc.transpose(2, 0, 1, 3).reshape(128, BL * G * DHW))
        sc = sbf[i * BL:(i + 1) * BL].reshape(BL, G, 128, SHW)
        spk = np.ascontiguousarray(
            sc.transpose(2, 0, 1, 3).reshape(128, BL * G * SHW))
        in_maps.append({"dpk": dpk, "spk": spk, "cpack": wp})
    return in_maps


def unpack_out(o):
    o = np.asarray(o).reshape(128, BL, G, SHW).transpose(1, 2, 0, 3)
    return o.reshape(BL, C, HS, WS).astype(np.float32)


def gather(results):
    return np.concatenate(
        [unpack_out(results[i]["out"]) for i in range(N_CORES)], axis=0)


def kernel(deeper, shallower, w1, b1, w2, b2):
    nc = _get_nc()
    in_maps = prepare_in_maps(deeper, shallower, w1, b1, w2, b2)
    res = bass_utils.run_bass_kernel_spmd(nc, in_maps, list(range(N_CORES)))
    return gather(res.results)
